# revision 4
# baseline (speedup 1.0000x reference)
"""Trainium2 Bass kernel for BeliefPlausibility (Dempster-Shafer bel/pl maps).

Problem: input [4, 384, 1248, 7] fp32 (6 singleton masses + omega per pixel).
Output: tuple (bel, pl), each [4, 384, 1248, 64] fp32 where, per pixel with
masses m_0..m_5 and omega w:
    bel[q] = sum_c m_c * ((q >> c) & 1)  for q in 1..62;  bel[0]=0, bel[63]=1
    pl[q]  = bel[q] + w                  for q in 1..62;  pl[0]=0,  pl[63]=1

Strategy (pure data parallel over 8 cores, no cross-device communication):
  - The kernel is HBM-write bound (~358 GB/s per core, 16 SDMA engines at
    ~23 GB/s each).  Everything runs in bf16 (inputs host-cast, outputs
    host-upcast); the 2e-2 relative-error budget dwarfs bf16's 2^-9
    rounding, and halving the output bytes halves the HBM-write floor.
  - Packed output: the 2 constant output columns per tensor (bel/pl 0 and
    63) never leave the device -- only 62 channels are written (-3.1%
    HBM bytes vs 64).  The host memsets the constants during unshard.
  - The weight matrix is laid out PSUM-packed: [112, 1008] with bel
    columns j*62+(q-1) for q=1..62 and the 16 omega lanes at columns
    992..1007.  The ACT PSUM->SBUF bel copy is then fully contiguous
    [128, 992], and the DVE pl = bel + omega add reads/writes contiguous
    SBUF with omega broadcast from PSUM via a zero-stride AP.
  - Output DRAM layout is partition-major [128, 117*992]: pixel block p,
    tile t, group j, channel q-1 lives at [p, t*992 + j*62 + q-1].  A
    grouped drain of 3 tiles is then ONE contiguous 5952 B descriptor
    per partition (32B-aligned, >=512B line-rate, 0.5% metadata), vs the
    strided 2048/1984 B descriptors of the channel-major layout.  The
    host untangles the (t, p) transpose during unshard (pure gather).
  - Each core gets 239,616 pixels = 117 supertiles of 2048 (128 blocks x
    16 groups).  The host pre-permutes its shard to lhsT layout
    [112, 117*128]; the whole shard is DMA'd into SBUF once (in 8 chunks
    so compute starts early) and sliced per supertile.
  - Per supertile: two bf16 matmuls -> PSUM [128, 1008] (fp32 exact),
    ACT copies bel, DVE adds pl, and every 3rd tile each staging buffer
    drains with one contiguous ~744 KB DMA per output tensor (bel on the
    sync HWDGE ring, pl on the scalar ring to split descriptor load).
    The last 6 tiles drain per-tile to shorten the pipeline tail.
"""

import sys

if "concourse" not in sys.modules:
    try:
        import concourse  # noqa: F401
    except ImportError:
        sys.path.insert(0, "/opt/trn_rl_repo")

import ml_dtypes
import numpy as np

import concourse.bacc as bacc
import concourse.bass as bass
import concourse.mybir as mybir
import concourse.tile as tile
from concourse.bass_utils import run_bass_kernel_spmd

F32 = mybir.dt.float32
BF16 = mybir.dt.bfloat16

N_CORES = 8
PX_TOTAL = 4 * 384 * 1248          # 1,916,928 pixels
PX_CORE = PX_TOTAL // N_CORES      # 239,616
PX_PART = 16                       # pixel groups per block (partition)
PX_TILE = 128 * PX_PART            # 2048 pixels per supertile
N_TILES = PX_CORE // PX_TILE       # 117
N_CH = 7                           # 6 singletons + omega
N_SUB = 64                         # output positions per pixel
N_PK = N_SUB - 2                   # 62 non-constant outputs per pixel
K_ROWS = PX_PART * N_CH            # 112 contraction rows
PK_W = PX_PART * N_PK              # 992 packed outputs per partition/tile
MM_W = PK_W + PX_PART              # 1008 PSUM cols per tile (bel + omega)
PS_STRIDE = 1024                   # PSUM tile stride (2-bank aligned)
N_PS = 4                           # PSUM bank-pair rotation depth
OUT_GRP = 3                        # supertiles per output staging buffer
N_OBUF = 8                         # output staging buffers (bel & pl each)
TAIL_TILES = 0                     # final tiles drained per-tile
CHUNKS = [15] * 8                  # input prefetch chunk sizes (tiles)


def _weight_matrix() -> np.ndarray:
    """[112, 1008]: W[7j+c, 62j+(q-1)] = (q>>c)&1 for q in 1..62, c in 0..5;
    W[7j+6, 992+j] = 1 (omega lane for the pl broadcast)."""
    w = np.zeros((K_ROWS, MM_W), np.float32)
    for j in range(PX_PART):
        for q in range(1, 63):
            for c in range(6):
                if (q >> c) & 1:
                    w[7 * j + c, N_PK * j + q - 1] = 1.0
        w[7 * j + 6, PK_W + j] = 1.0
    return w


def build_program(n_tiles: int = N_TILES, reps: int = 1,
                  out_grp: int = OUT_GRP) -> bass.Bass:
    # Bacc (not plain Bass): its compile() runs generate_event_semaphores,
    # which splits multi-semaphore waits into standalone event-sem
    # instructions (TRN2 allows at most one wait per instruction).
    nc = bacc.Bacc("TRN2")

    x = nc.dram_tensor("x", (K_ROWS, n_tiles * 128), BF16,
                       kind="ExternalInput")
    # Partition-major packed outputs: [block p, t*992 + j*62 + (q-1)]
    bel = nc.dram_tensor("bel", (128, n_tiles * PK_W), BF16,
                         kind="ExternalOutput")
    pl = nc.dram_tensor("pl", (128, n_tiles * PK_W), BF16,
                        kind="ExternalOutput")

    w_dram = nc.inline_tensor(
        _weight_matrix().astype(ml_dtypes.bfloat16), name="wmat")

    n_grp_tiles = n_tiles - TAIL_TILES
    assert n_grp_tiles % out_grp == 0
    gw = out_grp * PK_W

    with tile.TileContext(nc) as tc:
        with (
            tc.tile_pool(name="const", bufs=1) as cpool,
            tc.tile_pool(name="outb", bufs=1) as belpool,
            tc.tile_pool(name="outp", bufs=1) as plpool,
            tc.tile_pool(name="psM", bufs=1, space="PSUM") as psMpool,
        ):
            wmat = cpool.tile([K_ROWS, MM_W], BF16)
            nc.sync.dma_start(wmat[:], w_dram[:])
            # Chunked input prefetch: the tile framework tracks byte-range
            # deps, so matmul t only waits for its own chunk and compute
            # starts ~1 chunk into the load instead of after all 3.35 MB.
            x_all = cpool.tile([K_ROWS, n_tiles * 128], BF16)
            k = 0
            for ct in CHUNKS:
                if k >= n_tiles:
                    break
                cols = slice(k * 128, min(n_tiles, k + ct) * 128)
                nc.sync.dma_start(x_all[:, cols], x[:, cols])
                k += ct

            # Persistent slot-cycled tensors: PSUM bank pairs for the
            # matmuls, and bel/pl staging buffers of OUT_GRP supertiles.
            ps_all = psMpool.tile([128, N_PS * PS_STRIDE], F32)
            bel_all = belpool.tile([128, N_OBUF * gw], BF16)
            pl_all = plpool.tile([128, N_OBUF * gw], BF16)

            for it in range(reps * n_tiles):
                t = it % n_tiles
                grp, tt = divmod(t, out_grp)
                buf = grp % N_OBUF
                ps = ps_all[:, PS_STRIDE * (it % N_PS):
                            PS_STRIDE * (it % N_PS) + MM_W]
                lhsT = x_all[:, t * 128:(t + 1) * 128]
                off = buf * gw + tt * PK_W

                for lo, hi in ((0, 512), (512, MM_W)):
                    nc.tensor.matmul(ps[:, lo:hi], lhsT, wmat[:, lo:hi])

                # bel: ACT casts the packed PSUM block -> SBUF bf16,
                # fully contiguous [128, 992] on both sides.
                nc.scalar.copy(bel_all[:, off:off + PK_W], ps[:, 0:PK_W])

                # pl = bel + omega: omega lane j broadcast over its 62
                # channels straight from PSUM via a zero-stride AP.
                bel3 = bel_all[:, off:off + PK_W].rearrange(
                    "p (g q) -> p g q", q=N_PK)
                pl3 = pl_all[:, off:off + PK_W].rearrange(
                    "p (g q) -> p g q", q=N_PK)
                omc = ps[:, PK_W:MM_W].rearrange("p (g q) -> p g q", q=1)
                om = bass.AP(omc.tensor, omc.offset,
                             omc.ap[:-1] + [[0, N_PK]])
                nc.vector.tensor_add(pl3, bel3, om)

                last = reps * n_tiles - it <= TAIL_TILES and \
                    t >= n_grp_tiles
                if last:
                    # Tail drains per-tile so the final DMAs start as soon
                    # as each tile's data is ready.
                    dcols = slice(t * PK_W, (t + 1) * PK_W)
                    nc.sync.dma_start(bel[:, dcols],
                                      bel_all[:, off:off + PK_W])
                    # pl drains on the scalar engine's HWDGE ring so the
                    # two physical rings split the output descriptor load
                    nc.scalar.dma_start(pl[:, dcols],
                                        pl_all[:, off:off + PK_W])
                elif tt == out_grp - 1 and t < n_grp_tiles:
                    # One contiguous 5952 B descriptor per partition.
                    dcols = slice(grp * gw, (grp + 1) * gw)
                    scols = slice(buf * gw, (buf + 1) * gw)
                    nc.sync.dma_start(bel[:, dcols], bel_all[:, scols])
                    nc.scalar.dma_start(pl[:, dcols], pl_all[:, scols])

    nc.compile()
    return nc


_NC_CACHE: dict[int, bass.Bass] = {}


def _get_program(n_tiles: int) -> bass.Bass:
    if n_tiles not in _NC_CACHE:
        _NC_CACHE[n_tiles] = build_program(n_tiles)
    return _NC_CACHE[n_tiles]


def run_on_cores(x_flat: np.ndarray, **run_kwargs):
    """x_flat: [PX_TOTAL, 7] fp32. Returns (bel, pl) each [PX_TOTAL, 64]
    fp32, plus the raw BassKernelResults as third element."""
    nc = _get_program(N_TILES)
    in_maps = []
    for c in range(N_CORES):
        seg = x_flat[c * PX_CORE:(c + 1) * PX_CORE]
        # [t, blk, j, c] -> rows (j, c), cols (t, blk): lhsT layout
        x4 = seg.reshape(N_TILES, 128, PX_PART, N_CH)
        xp = x4.transpose(2, 3, 0, 1).reshape(K_ROWS, N_TILES * 128)
        in_maps.append({"x": np.ascontiguousarray(
            xp.astype(ml_dtypes.bfloat16))})
    rr = run_bass_kernel_spmd(nc, in_maps, core_ids=list(range(N_CORES)),
                              **run_kwargs)
    bel = np.empty((PX_TOTAL, N_SUB), np.float32)
    pl = np.empty((PX_TOTAL, N_SUB), np.float32)
    # constant columns never leave the device
    for arr in (bel, pl):
        arr[:, 0] = 0.0
        arr[:, 63] = 1.0
    for c, res in enumerate(rr.results):
        sl = slice(c * PX_CORE, (c + 1) * PX_CORE)
        for name, out in (("bel", bel), ("pl", pl)):
            # [p, t*992 + j*62 + qm] -> pixel (t*128+p)*16+j, channel qm+1
            arr = np.asarray(res[name]).reshape(128, N_TILES, PX_PART, N_PK)
            out[sl, 1:63] = arr.transpose(1, 0, 2, 3).reshape(PX_CORE, N_PK)
    return bel, pl, rr


def kernel(inputs: np.ndarray):
    inputs = np.ascontiguousarray(np.asarray(inputs, dtype=np.float32))
    b, hh, ww, ch = inputs.shape
    x_flat = inputs.reshape(-1, ch)
    bel, pl, _ = run_on_cores(x_flat)
    return (bel.reshape(b, hh, ww, N_SUB), pl.reshape(b, hh, ww, N_SUB))
